# revision 2
# baseline (speedup 1.0000x reference)
"""Trainium2 Bass kernel for nn_DecoupledTextDecoder.

Reference computation (per batch sample b, nB=256, nC=512, nH*nW=512, nT=40,
nCls=97):
  A_n   = A / sum_hw(A)                       (attention normalize)
  C     = einsum('chw,thw->tc', feature_b, A_n_b)       [40, 512]
  hidden= C @ W.T + b                                   [40, 512]
  cfP   = hidden @ protos.T                             [40, 97]
  cfCos = cfP / (||hidden||_row + EPS)
  outCls= concat([cfP * ALPHA, UNK], -1); outCos = concat([cfCos, UNK], -1)
  ragged-pack the first textLength[b] rows of each sample into one buffer.

Strategy: data-parallel over nB across 8 NeuronCores (32 samples/core).
The hw-contraction needs hw on SBUF partitions for the PE, so feature and A
are uploaded pre-transposed ([b, hw, c] / [b, hw, t]) — a host-side layout
choice — removing all on-chip transposes.  The attention normalization is
algebraically folded into a per-column scale s[t]=1/rowsum(A) applied to C
(host computes s exactly in fp32).  Per-sample matmul chain on the PE:
  M1: C^T[c,t]      = FT-chunks(lhsT) x AT-chunks(rhs), accumulated over hw
  M2: hidden^T[c',t]= WT-chunks(lhsT) x C^T(rhs),       accumulated over c
  M3: cfP[t,cls]    = hidden^T-slices(lhsT) x protos^T(rhs), accum over c'
  hnorm^2 via DVE squares + ones-matmul partition reduce -> [t,1] layout,
  so the cfCos division and ALPHA scale are per-partition tensor_scalars.
The ragged pack is pure data movement with runtime row offsets; it is done
on the host with a vectorized scatter (the per-sample rows shard cleanly).

Matmul operands use fp16 (inputs rounded on host / on copy); accumulation
is fp32 in PSUM and everything after M3 stays fp32.  Measured end-to-end
resid-var vs the fp32 reference is ~1e-7.
"""

import numpy as np

import concourse.bass as bass
import concourse.bacc as bacc
import concourse.tile as tile
import concourse.mybir as mybir
from concourse.bass_utils import run_bass_kernel_spmd

F32 = mybir.dt.float32
EPS = 0.0009

N_CORES = 8
NB = 256
NB_C = NB // N_CORES       # samples per core
NC = 512                   # channels
HW = 512                   # nH*nW
NT = 40                    # text steps
NCLS = 97
D = NCLS + 1

GROUP = 2                  # samples per feature DMA (1 MiB transfers)
BLOCKS = [12, 12, 8]       # samples per block (sum = NB_C)
assert sum(BLOCKS) == NB_C and all(b % GROUP == 0 for b in BLOCKS)


def _mgroups(ns):
    """Partition-dim groups of t-columns for M3 (<=128 each, 40-aligned)."""
    w = ns * NT
    out, o = [], 0
    while o < w:
        m = min(120, w - o)
        out.append((o, m))
        o += m
    return out


def build_kernel(dt2=mybir.dt.float16, reps=1):
    """Build + compile the per-core Bass program. Returns nc."""
    nc = bacc.Bacc("TRN2", target_bir_lowering=False, debug=False,
                   enable_asserts=True, num_devices=N_CORES)

    ft = nc.dram_tensor("ft", [NB_C * HW, NC], dt2, kind="ExternalInput").ap()
    at = nc.dram_tensor("at", [NB_C * HW, NT], dt2, kind="ExternalInput").ap()
    wt = nc.dram_tensor("wt", [NC, NC], dt2, kind="ExternalInput").ap()
    pt = nc.dram_tensor("pt", [NC, NCLS], dt2, kind="ExternalInput").ap()
    bcol = nc.dram_tensor("bcol", [128, NC // 128], F32, kind="ExternalInput").ap()
    srow = nc.dram_tensor("srow", [1, NB_C * NT], F32, kind="ExternalInput").ap()
    au = nc.dram_tensor("au", [1, 2], F32, kind="ExternalInput").ap()
    ocls = nc.dram_tensor("ocls", [NB_C * NT, D], F32, kind="ExternalOutput").ap()
    ocos = nc.dram_tensor("ocos", [NB_C * NT, D], F32, kind="ExternalOutput").ap()

    with tile.TileContext(nc) as tc:
        with (
            tc.tile_pool(name="consts", bufs=1) as consts,
            tc.tile_pool(name="ftp", bufs=3) as ftp,
            tc.tile_pool(name="atp", bufs=3) as atp,
            tc.tile_pool(name="work", bufs=8) as work,
            tc.tile_pool(name="sqp", bufs=5) as sqp,
            tc.tile_pool(name="outp", bufs=12) as outp,
            tc.tile_pool(name="ps_ct", bufs=4, space="PSUM") as ps_ct,
            tc.tile_pool(name="ps_h", bufs=2, space="PSUM") as ps_h,
            tc.tile_pool(name="ps_p", bufs=1, space="PSUM") as ps_p,
            tc.tile_pool(name="ps_sq", bufs=1, space="PSUM") as ps_sq,
        ):
            for _ in range(reps):
                _emit_once(nc, tc, consts, ftp, atp, work, sqp, outp,
                           ps_ct, ps_h, ps_p, ps_sq,
                           ft, at, wt, pt, bcol, srow, au, ocls, ocos, dt2)
    nc.compile()
    return nc


def _emit_once(nc, tc, consts, ftp, atp, work, sqp, outp,
               ps_ct, ps_h, ps_p, ps_sq,
               ft, at, wt, pt, bcol, srow, au, ocls, ocos, dt2):
    mult = mybir.AluOpType.mult
    add = mybir.AluOpType.add

    # ---- constants -------------------------------------------------------
    ones_row = consts.tile([1, 128], F32, tag="ones_row")
    nc.vector.memset(ones_row[:], 1.0)
    ones_col = consts.tile([128, 1], F32, tag="ones_col")
    nc.vector.memset(ones_col[:], 1.0)

    wt_sb = []
    for k in range(4):
        t = consts.tile([128, NC], dt2, tag=f"wt{k}")
        nc.sync.dma_start(out=t[:], in_=wt[k * 128:(k + 1) * 128, :])
        wt_sb.append(t)
    pt_sb = []
    for k in range(4):
        t = consts.tile([128, NCLS], dt2, tag=f"pt{k}")
        nc.sync.dma_start(out=t[:], in_=pt[k * 128:(k + 1) * 128, :])
        pt_sb.append(t)
    b_sb = consts.tile([128, 4], F32, tag="b")
    nc.sync.dma_start(out=b_sb[:], in_=bcol[:])
    s_sb = consts.tile([1, NB_C * NT], F32, tag="s")
    nc.sync.dma_start(out=s_sb[:], in_=srow[:])
    au_sb = consts.tile([1, 2], F32, tag="au")
    nc.sync.dma_start(out=au_sb[:], in_=au[:])

    # Broadcast s over partitions via k=1 matmul: S_all[p, col] = s[col].
    s_all = consts.tile([128, NB_C * NT], F32, tag="s_all")
    o = 0
    while o < NB_C * NT:
        w = min(512, NB_C * NT - o)
        ps = ps_h.tile([128, 512], F32, tag="h")
        nc.tensor.matmul(ps[:, :w], ones_row[:], s_sb[:, o:o + w],
                         start=True, stop=True)
        nc.vector.tensor_copy(s_all[:, o:o + w], ps[:, :w])
        o += w
    # alpha / unk broadcast columns
    au_ps = ps_h.tile([128, 512], F32, tag="h")
    nc.tensor.matmul(au_ps[:, :2], ones_row[:], au_sb[:], start=True, stop=True)
    au_col = consts.tile([128, 2], F32, tag="au_col")
    nc.vector.tensor_copy(au_col[:], au_ps[:, :2])
    alpha_col = au_col[:, 0:1]
    unk_col = au_col[:, 1:2]

    # ---- main loop over sample blocks -----------------------------------
    s0 = 0
    for ns in BLOCKS:
        w = ns * NT
        col0 = s0 * NT

        # feature/attention loads, GROUP samples per DMA
        ftg, atg = [], []
        for j in range(ns // GROUP):
            r0 = (s0 + j * GROUP) * HW
            ftile = ftp.tile([128, GROUP * 4, NC], dt2, tag="ft")
            nc.sync.dma_start(
                out=ftile[:],
                in_=ft[r0:r0 + GROUP * HW, :].rearrange("(g p) c -> p g c", p=128))
            ftg.append(ftile)
            atile = atp.tile([128, GROUP * 4, NT], dt2, tag="at")
            nc.sync.dma_start(
                out=atile[:],
                in_=at[r0:r0 + GROUP * HW, :].rearrange("(g p) t -> p g t", p=128))
            atg.append(atile)

        # M1: C_raw^T accumulated into 4 psum banks, one 40-col slice/sample
        ct_ps = [ps_ct.tile([128, 480], F32, tag="ct", name=f"ct_ps{jj}") for jj in range(4)]
        for sl in range(ns):
            ftile = ftg[sl // GROUP]
            atile = atg[sl // GROUP]
            h = sl % GROUP
            for jj in range(4):
                for kk in range(4):
                    nc.tensor.matmul(
                        ct_ps[jj][:, sl * NT:(sl + 1) * NT],
                        ftile[:, h * 4 + kk, jj * 128:(jj + 1) * 128],
                        atile[:, h * 4 + kk, :],
                        start=(kk == 0), stop=(kk == 3))

        # scale by s (normalization fold) + cast to dt2
        ct_sb = []
        for jj in range(4):
            t = work.tile([128, 480], dt2, tag="ctsb")
            nc.vector.tensor_tensor(t[:, :w], ct_ps[jj][:, :w],
                                    s_all[:, col0:col0 + w], mult)
            ct_sb.append(t)

        # M2: hidden^T (no bias yet), 4 psum banks
        h_sb = []
        sq = []
        for jj in range(4):
            hp = ps_h.tile([128, 480], F32, tag="h")
            for kk in range(4):
                nc.tensor.matmul(hp[:, :w], wt_sb[kk][:, jj * 128:(jj + 1) * 128],
                                 ct_sb[kk][:, :w], start=(kk == 0), stop=(kk == 3))
            # bias add on ACT during psum->sbuf copy (cast to dt2)
            hs = work.tile([128, 480], dt2, tag="hsb")
            nc.scalar.activation(hs[:, :w], hp[:, :w],
                                 mybir.ActivationFunctionType.Identity,
                                 bias=b_sb[:, jj:jj + 1])
            h_sb.append(hs)
            # squared hidden for the row norms
            st = sqp.tile([128, 480], F32, tag="sq")
            nc.vector.tensor_tensor(st[:, :w], hs[:, :w], hs[:, :w], mult)
            sq.append(st)
        nc.vector.tensor_tensor(sq[0][:, :w], sq[0][:, :w], sq[1][:, :w], add)
        nc.vector.tensor_tensor(sq[2][:, :w], sq[2][:, :w], sq[3][:, :w], add)
        nc.vector.tensor_tensor(sq[0][:, :w], sq[0][:, :w], sq[2][:, :w], add)

        mg = _mgroups(ns)
        # partition-reduce -> hnorm^2 in [t, 1] layout
        sq_ps = ps_sq.tile([128, len(mg)], F32, tag="sqc")
        for g, (o, m) in enumerate(mg):
            nc.tensor.matmul(sq_ps[:m, g:g + 1], sq[0][:, o:o + m], ones_col[:],
                             start=True, stop=True)
        # r = 1 / (sqrt(hnorm^2) + EPS)
        rcols = work.tile([128, len(mg)], F32, tag="rc")
        for g, (o, m) in enumerate(mg):
            nc.scalar.sqrt(rcols[:m, g:g + 1], sq_ps[:m, g:g + 1])
            nc.vector.tensor_scalar_add(rcols[:m, g:g + 1], rcols[:m, g:g + 1], EPS)
            nc.vector.reciprocal(rcols[:m, g:g + 1], rcols[:m, g:g + 1])

        # M3 + outputs
        p_ps = ps_p.tile([128, len(mg) * NCLS], F32, tag="p")
        for g, (o, m) in enumerate(mg):
            for kk in range(4):
                nc.tensor.matmul(p_ps[:m, g * NCLS:(g + 1) * NCLS],
                                 h_sb[kk][:, o:o + m], pt_sb[kk][:],
                                 start=(kk == 0), stop=(kk == 3))
            oc = outp.tile([128, D], F32, tag="ocls")
            nc.vector.tensor_scalar(oc[:m, 0:NCLS], p_ps[:m, g * NCLS:(g + 1) * NCLS],
                                    alpha_col[:m, :], None, mult)
            nc.vector.tensor_copy(oc[:m, NCLS:D], unk_col[:m, :])
            nc.sync.dma_start(out=ocls[col0 + o:col0 + o + m, :], in_=oc[:m, :])

            os_ = outp.tile([128, D], F32, tag="ocos")
            nc.vector.tensor_scalar(os_[:m, 0:NCLS], p_ps[:m, g * NCLS:(g + 1) * NCLS],
                                    rcols[:m, g:g + 1], None, mult)
            nc.vector.tensor_copy(os_[:m, NCLS:D], unk_col[:m, :])
            nc.sync.dma_start(out=ocos[col0 + o:col0 + o + m, :], in_=os_[:m, :])
        s0 += ns


def host_prep(feature, A, protos, W, b, ALPHA, UNK_SCR, np_dt=np.float16):
    """Build the 8 per-core input maps (host-side layout prep)."""
    f3 = np.ascontiguousarray(feature.reshape(NB, NC, HW).transpose(0, 2, 1)).astype(np_dt)
    a3r = A.reshape(NB, NT, HW)
    a3 = np.ascontiguousarray(a3r.transpose(0, 2, 1)).astype(np_dt)
    s = (1.0 / a3r.sum(axis=2, dtype=np.float64)).astype(np.float32)  # [NB, NT]
    wt = np.ascontiguousarray(W.T).astype(np_dt)
    pt = np.ascontiguousarray(protos.T).astype(np_dt)
    bcol = np.ascontiguousarray(b.reshape(4, 128).T).astype(np.float32)
    au = np.array([[float(ALPHA[0, 0]), float(UNK_SCR[0, 0])]], np.float32)
    in_maps = []
    for c in range(N_CORES):
        sl = slice(c * NB_C, (c + 1) * NB_C)
        in_maps.append(dict(
            ft=f3[sl].reshape(NB_C * HW, NC),
            at=a3[sl].reshape(NB_C * HW, NT),
            wt=wt, pt=pt, bcol=bcol,
            srow=s[sl].reshape(1, NB_C * NT),
            au=au,
        ))
    return in_maps


def host_pack(dense_cls, dense_cos, textLength):
    """Ragged per-sample packing (matches reference.pack)."""
    usedLen = np.minimum(textLength.astype(np.int64), NT)
    offsets = np.cumsum(usedLen) - usedLen
    b_idx, t_idx = np.nonzero(t_mask := (np.arange(NT)[None, :] < usedLen[:, None]))
    out_cls = np.zeros((NB * NT, D), np.float32)
    out_cos = np.zeros((NB * NT, D), np.float32)
    dest = offsets[b_idx] + t_idx
    src = b_idx * NT + t_idx
    out_cls[dest] = dense_cls[src]
    out_cos[dest] = dense_cos[src]
    return out_cls, out_cos


_NC_CACHE = {}


def _get_nc(dt2=mybir.dt.float16, reps=1):
    key = (str(dt2), reps)
    if key not in _NC_CACHE:
        _NC_CACHE[key] = build_kernel(dt2, reps)
    return _NC_CACHE[key]


def kernel(feature, A, protos, W, b, ALPHA, UNK_SCR, textLength):
    feature = np.asarray(feature, np.float32)
    A = np.asarray(A, np.float32)
    in_maps = host_prep(np.asarray(feature, np.float32), np.asarray(A, np.float32),
                        np.asarray(protos, np.float32), np.asarray(W, np.float32),
                        np.asarray(b, np.float32), np.asarray(ALPHA, np.float32),
                        np.asarray(UNK_SCR, np.float32))
    nc = _get_nc()
    res = run_bass_kernel_spmd(nc, in_maps, core_ids=list(range(N_CORES)))
    dense_cls = np.concatenate([res.results[c]["ocls"] for c in range(N_CORES)], axis=0)
    dense_cos = np.concatenate([res.results[c]["ocos"] for c in range(N_CORES)], axis=0)
    return host_pack(dense_cls, dense_cos, np.asarray(textLength))


# revision 26
# speedup vs baseline: 1.3223x; 1.3223x over previous
"""Trainium2 Bass kernel for nn_DecoupledTextDecoder.

Reference computation (per batch sample b, nB=256, nC=512, nH*nW=512, nT=40,
nCls=97):
  A_n   = A / sum_hw(A)                       (attention normalize)
  C     = einsum('chw,thw->tc', feature_b, A_n_b)       [40, 512]
  hidden= C @ W.T + b                                   [40, 512]
  cfP   = hidden @ protos.T                             [40, 97]
  cfCos = cfP / (||hidden||_row + EPS)
  outCls= concat([cfP * ALPHA, UNK], -1); outCos = concat([cfCos, UNK], -1)
  ragged-pack the first textLength[b] rows of each sample into one buffer.

Strategy: data-parallel over nB across 8 NeuronCores (32 samples/core).
The hw-contraction needs hw on SBUF partitions for the PE, so feature and A
are uploaded pre-transposed ([b, hw, c] / [b, hw, t]) — a host-side layout
choice — removing all on-chip transposes.  The attention normalization is
algebraically folded into a per-column scale s[t]=1/rowsum(A) applied to C
(host computes s exactly in fp32).  Per-sample matmul chain on the PE:
  M1: C^T[c,t]      = FT-chunks(lhsT) x AT-chunks(rhs), accumulated over hw
  M2: hidden^T[c',t]= WT-chunks(lhsT) x C^T(rhs),       accumulated over c
  M3: cfP[t,cls]    = hidden^T-slices(lhsT) x protos^T(rhs), accum over c'
  hnorm^2 via DVE squares + ones-matmul partition reduce -> [t,1] layout,
  so the cfCos division and ALPHA scale are per-partition tensor_scalars.
The ragged pack is pure data movement with runtime row offsets; it is done
on the host with a vectorized scatter (the per-sample rows shard cleanly).

Matmul operands use fp16 (inputs rounded on host / on copy); accumulation
is fp32 in PSUM and everything after M3 stays fp32.  Measured end-to-end
resid-var vs the fp32 reference is ~1e-7.
"""

import numpy as np

import concourse.bass as bass
import concourse.bacc as bacc
import concourse.tile as tile
import concourse.mybir as mybir
from concourse.bass_utils import run_bass_kernel_spmd

F32 = mybir.dt.float32
EPS = 0.0009

N_CORES = 8
NB = 256
NB_C = NB // N_CORES       # samples per core
NC = 512                   # channels
HW = 512                   # nH*nW
NT = 40                    # text steps
NCLS = 97
D = NCLS + 1

GROUP = 2                  # samples per feature DMA (1 MiB transfers)
BLOCKS = [12, 12, 8]       # samples per block (sum = NB_C)
assert sum(BLOCKS) == NB_C and all(b % GROUP == 0 for b in BLOCKS)


def _mgroups(ns):
    """Partition-dim groups of t-columns for M3 (<=128 each, 40-aligned)."""
    w = ns * NT
    out, o = [], 0
    while o < w:
        m = min(120, w - o)
        out.append((o, m))
        o += m
    return out


def build_kernel(dt2=mybir.dt.float16, reps=1, group=GROUP, dual_ring=False,
                 ft_bufs=3, timing_mode=False, hw_loop=0, rings=None,
                 out_rings=None, skip_load=False, skip_compute=False,
                 dup_dma=False, dup_m1=False, at_g=False, flat_ft=False):
    """Build + compile the per-core Bass program. Returns nc.

    timing_mode=True replaces the bulk inputs with Internal DRAM scratch so
    repeated-execution benchmarks don't pay host->device re-transfers; the
    on-device HBM traffic is identical.
    """
    nc = bacc.Bacc("TRN2", target_bir_lowering=False, debug=False,
                   enable_asserts=True, num_devices=N_CORES)

    kind_b = "Internal" if timing_mode else "ExternalInput"
    ft = nc.dram_tensor("ft", [NB_C * HW, NC], dt2, kind=kind_b).ap()
    at = nc.dram_tensor("at", [NB_C * HW, NT], dt2, kind=kind_b).ap()
    wt = nc.dram_tensor("wt", [NC, NC], dt2, kind=kind_b).ap()
    pt = nc.dram_tensor("pt", [NC, NCLS], dt2, kind=kind_b).ap()
    bcol = nc.dram_tensor("bcol", [128, NC // 128], F32, kind=kind_b).ap()
    srow = nc.dram_tensor("srow", [1, NB_C * NT], F32, kind=kind_b).ap()
    au = nc.dram_tensor("au", [1, 2], F32, kind="ExternalInput").ap()
    ocls = nc.dram_tensor("ocls", [NB_C * NT, D], F32, kind="ExternalOutput").ap()
    ocos = nc.dram_tensor("ocos", [NB_C * NT, D], F32, kind="ExternalOutput").ap()

    with tile.TileContext(nc) as tc:
        with (
            tc.tile_pool(name="consts", bufs=1) as consts,
            tc.tile_pool(name="ftp", bufs=ft_bufs) as ftp,
            tc.tile_pool(name="atp", bufs=3) as atp,
            tc.tile_pool(name="work", bufs=8) as work,
            tc.tile_pool(name="sqp", bufs=5) as sqp,
            tc.tile_pool(name="outp", bufs=12) as outp,
            tc.tile_pool(name="ps_ct", bufs=4, space="PSUM") as ps_ct,
            tc.tile_pool(name="ps_h", bufs=2, space="PSUM") as ps_h,
            tc.tile_pool(name="ps_p", bufs=1, space="PSUM") as ps_p,
            tc.tile_pool(name="ps_sq", bufs=1, space="PSUM") as ps_sq,
        ):
            if rings is None:
                ring_eng = [nc.scalar, nc.sync] if dual_ring else [nc.sync]
            else:
                emap = {"s": nc.sync, "a": nc.scalar, "g": nc.gpsimd}
                ring_eng = [emap[ch] for ch in rings]
            if out_rings is None:
                oring_eng = [nc.sync]
            else:
                emap = {"s": nc.sync, "a": nc.scalar, "g": nc.gpsimd}
                oring_eng = [emap[ch] for ch in out_rings]

            def emit():
                _emit_once(nc, tc, consts, ftp, atp, work, sqp, outp,
                           ps_ct, ps_h, ps_p, ps_sq,
                           ft, at, wt, pt, bcol, srow, au, ocls, ocos, dt2,
                           group, ring_eng, oring_eng, skip_load, skip_compute,
                           dup_dma, dup_m1, at_g, flat_ft)

            if hw_loop:
                with tc.For_i(0, hw_loop, 1):
                    emit()
            else:
                for _ in range(reps):
                    emit()
    nc.compile()
    return nc


def _emit_once(nc, tc, consts, ftp, atp, work, sqp, outp,
               ps_ct, ps_h, ps_p, ps_sq,
               ft, at, wt, pt, bcol, srow, au, ocls, ocos, dt2,
               group, ring_eng, oring_eng, skip_load=False, skip_compute=False,
               dup_dma=False, dup_m1=False, at_g=False, flat_ft=False):
    mult = mybir.AluOpType.mult
    add = mybir.AluOpType.add

    # ---- constants -------------------------------------------------------
    ones_row = consts.tile([1, 128], F32, tag="ones_row")
    nc.vector.memset(ones_row[:], 1.0)
    ones_col = consts.tile([128, 1], F32, tag="ones_col")
    nc.vector.memset(ones_col[:], 1.0)

    wt_sb = []
    for k in range(4):
        t = consts.tile([128, NC], dt2, tag=f"wt{k}")
        nc.sync.dma_start(out=t[:], in_=wt[k * 128:(k + 1) * 128, :])
        wt_sb.append(t)
    pt_sb = []
    for k in range(4):
        t = consts.tile([128, NCLS], dt2, tag=f"pt{k}")
        nc.sync.dma_start(out=t[:], in_=pt[k * 128:(k + 1) * 128, :])
        pt_sb.append(t)
    b_sb = consts.tile([128, 4], F32, tag="b")
    nc.sync.dma_start(out=b_sb[:], in_=bcol[:])
    s_sb = consts.tile([1, NB_C * NT], F32, tag="s")
    nc.sync.dma_start(out=s_sb[:], in_=srow[:])
    au_sb = consts.tile([1, 2], F32, tag="au")
    nc.sync.dma_start(out=au_sb[:], in_=au[:])

    # Broadcast s over partitions via k=1 matmul: S_all[p, col] = s[col].
    s_all = consts.tile([128, NB_C * NT], F32, tag="s_all")
    o = 0
    while o < NB_C * NT:
        w = min(512, NB_C * NT - o)
        ps = ps_h.tile([128, 512], F32, tag="h")
        nc.tensor.matmul(ps[:, :w], ones_row[:], s_sb[:, o:o + w],
                         start=True, stop=True)
        nc.vector.tensor_copy(s_all[:, o:o + w], ps[:, :w])
        o += w
    # alpha / unk broadcast columns
    au_ps = ps_h.tile([128, 512], F32, tag="h")
    nc.tensor.matmul(au_ps[:, :2], ones_row[:], au_sb[:], start=True, stop=True)
    au_col = consts.tile([128, 2], F32, tag="au_col")
    nc.vector.tensor_copy(au_col[:], au_ps[:, :2])
    alpha_col = au_col[:, 0:1]
    unk_col = au_col[:, 1:2]

    # ---- main loop over sample blocks -----------------------------------
    s0 = 0
    for ns in BLOCKS:
        w = ns * NT
        col0 = s0 * NT

        # feature/attention loads, `group` samples per DMA
        ftg, atg = [], []
        for j in range(ns // group):
            r0 = (s0 + j * group) * HW
            eng = ring_eng[j % len(ring_eng)]
            ftile = ftp.tile([128, group * 4, NC], dt2, tag="ft")
            atile = atp.tile([128, group * 4, NT], dt2, tag="at")
            if not skip_load:
                if flat_ft:
                    for q in range(group * 4):
                        ring_eng[(j * group * 4 + q) % len(ring_eng)].dma_start(
                            out=ftile[:, q, :],
                            in_=ft[r0 + q * 128:r0 + (q + 1) * 128, :])
                else:
                    eng.dma_start(
                        out=ftile[:],
                        in_=ft[r0:r0 + group * HW, :].rearrange("(g p) c -> p g c", p=128))
                (nc.gpsimd if at_g else eng).dma_start(
                    out=atile[:],
                    in_=at[r0:r0 + group * HW, :].rearrange("(g p) t -> p g t", p=128))
                if dup_dma:
                    dtile = ftp.tile([128, group * 4, NC], dt2, tag="ftdup", name="dtile")
                    eng.dma_start(
                        out=dtile[:],
                        in_=ft[r0:r0 + group * HW, :].rearrange("(g p) c -> p g c", p=128))
            ftg.append(ftile)
            atg.append(atile)
        if skip_compute:
            s0 += ns
            continue

        # M1: C_raw^T accumulated into 4 psum banks, one 40-col slice/sample
        ct_ps = [ps_ct.tile([128, 480], F32, tag="ct", name=f"ct_ps{jj}") for jj in range(4)]
        for sl in range(ns):
            ftile = ftg[sl // group]
            atile = atg[sl // group]
            h = sl % group
            for rep2 in range(2 if dup_m1 else 1):
                for jj in range(4):
                    for kk in range(4):
                        nc.tensor.matmul(
                            ct_ps[jj][:, sl * NT:(sl + 1) * NT],
                            ftile[:, h * 4 + kk, jj * 128:(jj + 1) * 128],
                            atile[:, h * 4 + kk, :],
                            start=(kk == 0), stop=(kk == 3))

        # scale by s (normalization fold) + cast to dt2
        ct_sb = []
        for jj in range(4):
            t = work.tile([128, 480], dt2, tag="ctsb")
            nc.vector.tensor_tensor(t[:, :w], ct_ps[jj][:, :w],
                                    s_all[:, col0:col0 + w], mult)
            ct_sb.append(t)

        # M2: hidden^T (no bias yet), 4 psum banks
        h_sb = []
        sq = []
        for jj in range(4):
            hp = ps_h.tile([128, 480], F32, tag="h")
            for kk in range(4):
                nc.tensor.matmul(hp[:, :w], wt_sb[kk][:, jj * 128:(jj + 1) * 128],
                                 ct_sb[kk][:, :w], start=(kk == 0), stop=(kk == 3))
            # bias add on ACT during psum->sbuf copy (cast to dt2)
            hs = work.tile([128, 480], dt2, tag="hsb")
            nc.scalar.activation(hs[:, :w], hp[:, :w],
                                 mybir.ActivationFunctionType.Identity,
                                 bias=b_sb[:, jj:jj + 1])
            h_sb.append(hs)
            # squared hidden for the row norms
            st = sqp.tile([128, 480], F32, tag="sq")
            nc.vector.tensor_tensor(st[:, :w], hs[:, :w], hs[:, :w], mult)
            sq.append(st)
        nc.vector.tensor_tensor(sq[0][:, :w], sq[0][:, :w], sq[1][:, :w], add)
        nc.vector.tensor_tensor(sq[2][:, :w], sq[2][:, :w], sq[3][:, :w], add)
        nc.vector.tensor_tensor(sq[0][:, :w], sq[0][:, :w], sq[2][:, :w], add)

        mg = _mgroups(ns)
        # partition-reduce -> hnorm^2 in [t, 1] layout
        sq_ps = ps_sq.tile([128, len(mg)], F32, tag="sqc")
        for g, (o, m) in enumerate(mg):
            nc.tensor.matmul(sq_ps[:m, g:g + 1], sq[0][:, o:o + m], ones_col[:],
                             start=True, stop=True)
        # r = 1 / (sqrt(hnorm^2) + EPS)
        rcols = work.tile([128, len(mg)], F32, tag="rc")
        for g, (o, m) in enumerate(mg):
            nc.scalar.sqrt(rcols[:m, g:g + 1], sq_ps[:m, g:g + 1])
            nc.vector.tensor_scalar_add(rcols[:m, g:g + 1], rcols[:m, g:g + 1], EPS)
            nc.vector.reciprocal(rcols[:m, g:g + 1], rcols[:m, g:g + 1])

        # M3 + outputs
        p_ps = ps_p.tile([128, len(mg) * NCLS], F32, tag="p")
        for g, (o, m) in enumerate(mg):
            for kk in range(4):
                nc.tensor.matmul(p_ps[:m, g * NCLS:(g + 1) * NCLS],
                                 h_sb[kk][:, o:o + m], pt_sb[kk][:],
                                 start=(kk == 0), stop=(kk == 3))
            oc = outp.tile([128, D], F32, tag="ocls")
            nc.vector.tensor_scalar(oc[:m, 0:NCLS], p_ps[:m, g * NCLS:(g + 1) * NCLS],
                                    alpha_col[:m, :], None, mult)
            nc.vector.tensor_copy(oc[:m, NCLS:D], unk_col[:m, :])
            oring_eng[g % len(oring_eng)].dma_start(
                out=ocls[col0 + o:col0 + o + m, :], in_=oc[:m, :])

            os_ = outp.tile([128, D], F32, tag="ocos")
            nc.vector.tensor_scalar(os_[:m, 0:NCLS], p_ps[:m, g * NCLS:(g + 1) * NCLS],
                                    rcols[:m, g:g + 1], None, mult)
            nc.vector.tensor_copy(os_[:m, NCLS:D], unk_col[:m, :])
            oring_eng[(g + 1) % len(oring_eng)].dma_start(
                out=ocos[col0 + o:col0 + o + m, :], in_=os_[:m, :])
        s0 += ns


def host_prep(feature, A, protos, W, b, ALPHA, UNK_SCR, np_dt=np.float16):
    """Build the 8 per-core input maps (host-side layout prep)."""
    f3 = np.ascontiguousarray(feature.reshape(NB, NC, HW).transpose(0, 2, 1)).astype(np_dt)
    a3r = A.reshape(NB, NT, HW)
    a3 = np.ascontiguousarray(a3r.transpose(0, 2, 1)).astype(np_dt)
    s = (1.0 / a3r.sum(axis=2, dtype=np.float64)).astype(np.float32)  # [NB, NT]
    wt = np.ascontiguousarray(W.T).astype(np_dt)
    pt = np.ascontiguousarray(protos.T).astype(np_dt)
    bcol = np.ascontiguousarray(b.reshape(4, 128).T).astype(np.float32)
    au = np.array([[float(ALPHA[0, 0]), float(UNK_SCR[0, 0])]], np.float32)
    in_maps = []
    for c in range(N_CORES):
        sl = slice(c * NB_C, (c + 1) * NB_C)
        in_maps.append(dict(
            ft=f3[sl].reshape(NB_C * HW, NC),
            at=a3[sl].reshape(NB_C * HW, NT),
            wt=wt, pt=pt, bcol=bcol,
            srow=s[sl].reshape(1, NB_C * NT),
            au=au,
        ))
    return in_maps


def host_pack(dense_cls, dense_cos, textLength):
    """Ragged per-sample packing (matches reference.pack)."""
    usedLen = np.minimum(textLength.astype(np.int64), NT)
    offsets = np.cumsum(usedLen) - usedLen
    b_idx, t_idx = np.nonzero(t_mask := (np.arange(NT)[None, :] < usedLen[:, None]))
    out_cls = np.zeros((NB * NT, D), np.float32)
    out_cos = np.zeros((NB * NT, D), np.float32)
    dest = offsets[b_idx] + t_idx
    src = b_idx * NT + t_idx
    out_cls[dest] = dense_cls[src]
    out_cos[dest] = dense_cos[src]
    return out_cls, out_cos


_NC_CACHE = {}


def _get_nc(dt2=mybir.dt.float16, reps=1, **kw):
    key = (str(dt2), reps, tuple(sorted(kw.items())))
    if key not in _NC_CACHE:
        _NC_CACHE[key] = build_kernel(dt2, reps, **kw)
    return _NC_CACHE[key]


FINAL_CFG = dict(dual_ring=True, out_rings="sa")


def kernel(feature, A, protos, W, b, ALPHA, UNK_SCR, textLength):
    feature = np.asarray(feature, np.float32)
    A = np.asarray(A, np.float32)
    in_maps = host_prep(np.asarray(feature, np.float32), np.asarray(A, np.float32),
                        np.asarray(protos, np.float32), np.asarray(W, np.float32),
                        np.asarray(b, np.float32), np.asarray(ALPHA, np.float32),
                        np.asarray(UNK_SCR, np.float32))
    nc = _get_nc(**FINAL_CFG)
    res = run_bass_kernel_spmd(nc, in_maps, core_ids=list(range(N_CORES)))
    dense_cls = np.concatenate([res.results[c]["ocls"] for c in range(N_CORES)], axis=0)
    dense_cos = np.concatenate([res.results[c]["ocos"] for c in range(N_CORES)], axis=0)
    return host_pack(dense_cls, dense_cos, np.asarray(textLength))
